# revision 5
# baseline (speedup 1.0000x reference)
"""InfoNCE loss (nn_InfoNCELoss) on 8 Trainium2 NeuronCores.

reference math:
    logits = (F @ F.T) / T                        # [N, N], T = 0.05
    mask   = labels[:, None] == labels[None, :]
    denom  = sum(exp(logits) * mask, axis=1)      # [N]
    loss   = -mean(logits - log(denom)[:, None])
         = -( sum(logits)/N^2 - sum(log(denom))/N )

Sharding: rows of F are split across 8 cores (1024 rows each). Every core
streams the full F^T (the "all-gathered" copy, prepared host-side as a
layout transform) and computes its 1024 x 8192 tile of the similarity
matrix with fp32r matmuls, applies exp on the scalar engine, multiplies by
the label-equality mask and row-reduces on the vector engine (fused
tensor_tensor_reduce). sum(logits) comes from per-block column sums folded
through the same lhsT via tiny extra matmuls. Per-core partials (log-denoms
per row + block sums of raw S) are combined into the scalar loss on host.
"""

from contextlib import ExitStack

import numpy as np

N = 8192          # total rows
D = 1024          # feature dim
NCORES = 8
NS = N // NCORES  # rows per core (1024)
KT = D // 128     # contraction k-tiles (8)
MT = NS // 128    # output m-tiles per core (8)
NB = 512          # n-block (free dim per matmul / psum bank)
NBLK = N // NB    # n-blocks (16)
INV_T = 20.0      # 1 / 0.05

_cached = {}


def _build_nc():
    import concourse.bacc as bacc
    import concourse.mybir as mybir
    import concourse.tile as tile

    f32 = mybir.dt.float32
    f32r = mybir.dt.float32r

    nc = bacc.Bacc("TRN2", target_bir_lowering=False, debug=False,
                   num_devices=NCORES)

    ft = nc.dram_tensor("ft", [D, N], f32r, kind="ExternalInput")
    own = nc.dram_tensor("own", [D, NS], f32r, kind="ExternalInput")
    clab = nc.dram_tensor("clab", [1, N], f32, kind="ExternalInput")
    rlab = nc.dram_tensor("rlab", [NS], f32, kind="ExternalInput")
    denom = nc.dram_tensor("denom", [128, MT], f32, kind="ExternalOutput")
    t1 = nc.dram_tensor("t1", [128, MT, NBLK], f32, kind="ExternalOutput")

    ft_r = ft.ap().rearrange("(k p) n -> p k n", p=128)
    own_r = own.ap().rearrange("(k p) m -> p k m", p=128)
    rlab_r = rlab.ap().rearrange("(m p) -> p m", p=128)

    Exp = mybir.ActivationFunctionType.Exp
    Copy = mybir.ActivationFunctionType.Copy
    add = mybir.AluOpType.add
    mult = mybir.AluOpType.mult
    is_equal = mybir.AluOpType.is_equal
    AX = mybir.AxisListType.X

    with tile.TileContext(nc) as tc, ExitStack() as ctx:
        singles = ctx.enter_context(tc.tile_pool(name="singles", bufs=1))
        rhs_pool = ctx.enter_context(tc.tile_pool(name="rhs_pool", bufs=3))
        work = ctx.enter_context(tc.tile_pool(name="work", bufs=3))
        psum = ctx.enter_context(tc.tile_pool(name="psum", bufs=6, space="PSUM"))
        accs = ctx.enter_context(tc.tile_pool(name="accs", bufs=1))

        # resident tiles
        own_t = singles.tile([128, KT, NS], f32r)
        nc.sync.dma_start(out=own_t, in_=own_r)
        clab_t = singles.tile([128, N], f32)
        nc.sync.dma_start(out=clab_t, in_=clab.ap().to_broadcast([128, N]))
        rlab_t = singles.tile([128, MT], f32)
        nc.sync.dma_start(out=rlab_t, in_=rlab_r)

        denom_cols = accs.tile([128, MT, NBLK], f32)
        rsum = accs.tile([128, KT, NBLK], f32)

        for b in range(NBLK):
            rhs_t = rhs_pool.tile([128, KT, NB], f32r, tag="rhs")
            nc.sync.dma_start(out=rhs_t, in_=ft_r[:, :, b * NB:(b + 1) * NB])
            # column sums of this block of F^T (feeds sum(logits) matmuls)
            nc.vector.tensor_reduce(out=rsum[:, :, b], in_=rhs_t.bitcast(f32),
                                    axis=AX, op=add)
            for m in range(MT):
                ps = psum.tile([128, NB], f32, tag="ps")
                for k in range(KT):
                    nc.tensor.matmul(
                        ps,
                        lhsT=own_t[:, k, m * 128:(m + 1) * 128],
                        rhs=rhs_t[:, k, :],
                        start=(k == 0),
                        stop=(k == KT - 1),
                    )
                ex = work.tile([128, NB], f32, tag="ex")
                nc.scalar.activation(ex, ps, Exp, scale=INV_T)
                # fused mask + multiply + row-sum:
                #   tr = (clab == rlab[m]) * ex ; denom_cols[...] = sum(tr)
                tr = work.tile([128, NB], f32, tag="tr")
                nc.vector.scalar_tensor_tensor(
                    out=tr,
                    in0=clab_t[:, b * NB:(b + 1) * NB],
                    scalar=rlab_t[:, m:m + 1],
                    in1=ex,
                    op0=is_equal,
                    op1=mult,
                    accum_out=denom_cols[:, m, b:b + 1],
                )

        # sum(logits): t1[p, m, b] = sum_d own[d, m*128+p] * rsum[d, b]
        for m in range(MT):
            pt = psum.tile([128, NBLK], f32, tag="pt", bufs=2)
            for k in range(KT):
                nc.tensor.matmul(
                    pt,
                    lhsT=own_t[:, k, m * 128:(m + 1) * 128].bitcast(f32),
                    rhs=rsum[:, k, :],
                    start=(k == 0),
                    stop=(k == KT - 1),
                )
            o1 = work.tile([128, NBLK], f32, tag="o1")
            nc.scalar.activation(o1, pt, Copy)
            nc.sync.dma_start(out=t1.ap()[:, m, :], in_=o1)

        denom_acc = accs.tile([128, MT], f32)
        nc.vector.tensor_reduce(out=denom_acc, in_=denom_cols, axis=AX, op=add)
        nc.sync.dma_start(out=denom.ap(), in_=denom_acc)

    nc.compile()
    return nc


def _get_nc():
    if "nc" not in _cached:
        _cached["nc"] = _build_nc()
    return _cached["nc"]


def kernel(features, labels):
    from concourse.bass_utils import run_bass_kernel_spmd

    features = np.ascontiguousarray(np.asarray(features, dtype=np.float32))
    labf = np.asarray(labels).astype(np.float32)

    ftT = np.ascontiguousarray(features.T)  # [D, N]
    in_maps = []
    for c in range(NCORES):
        in_maps.append({
            "ft": ftT,
            "own": np.ascontiguousarray(ftT[:, c * NS:(c + 1) * NS]),
            "clab": labf.reshape(1, N),
            "rlab": np.ascontiguousarray(labf[c * NS:(c + 1) * NS]),
        })

    nc = _get_nc()
    trace = bool(_cached.get("trace", False))
    res = run_bass_kernel_spmd(nc, in_maps, core_ids=list(range(NCORES)),
                               trace=trace)
    _cached["last_results"] = res

    with np.errstate(all="ignore"):
        sum_S = np.float64(0.0)
        sum_logd = np.float64(0.0)
        for c in range(NCORES):
            sum_S += np.float32(res.results[c]["t1"].sum(dtype=np.float64))
            sum_logd += np.float32(
                np.log(res.results[c]["denom"].astype(np.float64)).sum())
        loss = -(INV_T * sum_S / (N * N) - sum_logd / N)
    return np.float32(loss)


# revision 6
# speedup vs baseline: 1.2401x; 1.2401x over previous
"""InfoNCE loss (nn_InfoNCELoss) on 8 Trainium2 NeuronCores.

reference math:
    logits = (F @ F.T) / T                        # [N, N], T = 0.05
    mask   = labels[:, None] == labels[None, :]
    denom  = sum(exp(logits) * mask, axis=1)      # [N]
    loss   = -mean(logits - log(denom)[:, None])
         = -( sum(logits)/N^2 - sum(log(denom))/N )

Sharding: rows of F are split across 8 cores (1024 rows each). Every core
streams the full F^T (the "all-gathered" copy, prepared host-side as a
layout transform) and computes its 1024 x 8192 tile of the similarity
matrix on the tensor engine, applies exp on the scalar engine, and applies
the label-equality mask + multiply + row-reduce in a single fused
scalar_tensor_tensor on the vector engine. sum(logits) uses the closed form
sum_ij f_i.f_j = ||sum_i f_i||^2: each core reduces its own feature rows
once; the [1024]-vector partials and the per-row masked-exp denominators
are combined into the scalar loss on host (log of 8192 values + dot).
"""

from contextlib import ExitStack

import numpy as np

N = 8192          # total rows
D = 1024          # feature dim
NCORES = 8
NS = N // NCORES  # rows per core (1024)
KT = D // 128     # contraction k-tiles (8)
MT = NS // 128    # output m-tiles per core (8)
NB = 512          # n-block (free dim per matmul / psum bank)
NBLK = N // NB    # n-blocks (16)
INV_T = 20.0      # 1 / 0.05

_cached = {}


def _build_nc():
    import concourse.bacc as bacc
    import concourse.mybir as mybir
    import concourse.tile as tile

    f32 = mybir.dt.float32
    bf16 = mybir.dt.bfloat16

    nc = bacc.Bacc("TRN2", target_bir_lowering=False, debug=False,
                   num_devices=NCORES)

    ft = nc.dram_tensor("ft", [D, N], bf16, kind="ExternalInput")
    own = nc.dram_tensor("own", [D, NS], bf16, kind="ExternalInput")
    clab = nc.dram_tensor("clab", [1, N], bf16, kind="ExternalInput")
    rlab = nc.dram_tensor("rlab", [NS], bf16, kind="ExternalInput")
    denom = nc.dram_tensor("denom", [128, MT], f32, kind="ExternalOutput")
    srow = nc.dram_tensor("srow", [128, KT], f32, kind="ExternalOutput")

    ft_r = ft.ap().rearrange("(k p) n -> p k n", p=128)
    own_r = own.ap().rearrange("(k p) m -> p k m", p=128)
    rlab_r = rlab.ap().rearrange("(m p) -> p m", p=128)

    Exp = mybir.ActivationFunctionType.Exp
    add = mybir.AluOpType.add
    mult = mybir.AluOpType.mult
    is_equal = mybir.AluOpType.is_equal
    AX = mybir.AxisListType.X

    with tile.TileContext(nc) as tc, ExitStack() as ctx:
        singles = ctx.enter_context(tc.tile_pool(name="singles", bufs=1))
        rhs_pool = ctx.enter_context(tc.tile_pool(name="rhs_pool", bufs=3))
        work = ctx.enter_context(tc.tile_pool(name="work", bufs=3))
        psum = ctx.enter_context(tc.tile_pool(name="psum", bufs=8, space="PSUM"))
        accs = ctx.enter_context(tc.tile_pool(name="accs", bufs=1))

        # resident tiles; split the startup DMAs per k-tile so the first
        # matmuls only wait for their own slices
        own_t = singles.tile([128, KT, NS], bf16)
        rhs0 = rhs_pool.tile([128, KT, NB], bf16, tag="rhs")
        for k in range(KT):
            nc.sync.dma_start(out=own_t[:, k, :], in_=own_r[:, k, :])
            nc.sync.dma_start(out=rhs0[:, k, :], in_=ft_r[:, k, 0:NB])
        clab_t = singles.tile([128, N], bf16)
        nc.sync.dma_start(out=clab_t, in_=clab.ap().to_broadcast([128, N]))
        rlab_t = singles.tile([128, MT], bf16)
        nc.sync.dma_start(out=rlab_t, in_=rlab_r)

        denom_cols = accs.tile([128, MT, NBLK], f32)

        for b in range(NBLK):
            if b == 0:
                rhs_t = rhs0
            else:
                rhs_t = rhs_pool.tile([128, KT, NB], bf16, tag="rhs")
                nc.sync.dma_start(out=rhs_t,
                                  in_=ft_r[:, :, b * NB:(b + 1) * NB])
            for m in range(MT):
                ps = psum.tile([128, NB], f32, tag="ps")
                for k in range(KT):
                    nc.tensor.matmul(
                        ps,
                        lhsT=own_t[:, k, m * 128:(m + 1) * 128],
                        rhs=rhs_t[:, k, :],
                        start=(k == 0),
                        stop=(k == KT - 1),
                    )
                ex = work.tile([128, NB], bf16, tag="ex")
                nc.scalar.activation(ex, ps, Exp, scale=INV_T)
                # fused mask + multiply + row-sum:
                #   tr = (clab == rlab[m]) * ex ; denom_cols[:, m, b] = sum(tr)
                tr = work.tile([128, NB], bf16, tag="tr")
                nc.vector.scalar_tensor_tensor(
                    out=tr,
                    in0=clab_t[:, b * NB:(b + 1) * NB],
                    scalar=rlab_t[:, m:m + 1],
                    in1=ex,
                    op0=is_equal,
                    op1=mult,
                    accum_out=denom_cols[:, m, b:b + 1],
                )

        # per-core row-sum of own features: srow[p, k] = sum_rows F[row, k*128+p]
        srow_t = accs.tile([128, KT], f32)
        nc.vector.tensor_reduce(out=srow_t, in_=own_t, axis=AX, op=add)
        nc.sync.dma_start(out=srow.ap(), in_=srow_t)

        denom_acc = accs.tile([128, MT], f32)
        nc.vector.tensor_reduce(out=denom_acc, in_=denom_cols, axis=AX, op=add)
        nc.sync.dma_start(out=denom.ap(), in_=denom_acc)

    nc.compile()
    return nc


def _get_nc():
    if "nc" not in _cached:
        _cached["nc"] = _build_nc()
    return _cached["nc"]


def kernel(features, labels):
    import ml_dtypes
    from concourse.bass_utils import run_bass_kernel_spmd

    bf16 = ml_dtypes.bfloat16
    features = np.ascontiguousarray(np.asarray(features, dtype=np.float32))
    labf = np.asarray(labels).astype(bf16)

    ftT = np.ascontiguousarray(features.T.astype(bf16))  # [D, N] bf16
    in_maps = []
    for c in range(NCORES):
        in_maps.append({
            "ft": ftT,
            "own": np.ascontiguousarray(ftT[:, c * NS:(c + 1) * NS]),
            "clab": labf.reshape(1, N),
            "rlab": np.ascontiguousarray(labf[c * NS:(c + 1) * NS]),
        })

    nc = _get_nc()
    trace = bool(_cached.get("trace", False))
    res = run_bass_kernel_spmd(nc, in_maps, core_ids=list(range(NCORES)),
                               trace=trace)
    _cached["last_results"] = res

    with np.errstate(all="ignore"):
        # sum(logits) = (1/T) * ||sum_i f_i||^2 via per-core partials
        s_total = np.zeros((128, KT), np.float64)
        sum_logd = np.float64(0.0)
        for c in range(NCORES):
            s_total += res.results[c]["srow"].astype(np.float64)
            sum_logd += np.float32(
                np.log(res.results[c]["denom"].astype(np.float64)).sum())
        sum_S = float(np.float32((s_total * s_total).sum()))
        loss = -(INV_T * sum_S / (N * N) - sum_logd / N)
    return np.float32(loss)
